# revision 18
# baseline (speedup 1.0000x reference)
"""Trainium2 Bass kernel for nn_Attention_21715354649378.

Reference computation (per batch b of 4):
    qkv = w_qkv @ x        x: [256, 4096(=64x64)]   w_qkv: [384, 256]
    q,k,v: [4 heads, 32, 4096];  q *= 32**-0.5
    sim_h = q_h^T k_h   [4096, 4096];  attn = softmax(sim, axis=-1)
    out_h = attn @ v_h^T    -> [4096, 32]
    out = w_out @ concat_heads + b_out   [256, 4096]

Sharding: 8 cores = 4 batches x 2 query-halves. Each core computes K/V for
its full batch plus attention + output projection for its half of the query
pixels. Outputs are disjoint slices -> no collectives.

Device algorithm per core (keys-in-partition layout, no max-subtraction --
sim values are O(6) so exp is safe in f32):
    vT = x^T W_v^T          per 128-key tile, 4 head blocks [v_h (32) | 1]
    krep_h = repl4(W_k,h) x   [128 = 4 copies of k_h(32d), 4096]  bf16
    qrep_h = repl4(s W_q,h) xq [128, 2048] bf16
      (replication lets QK use PE row-group kt%4: fast-weight-loads overlap
       and up to 4 concurrent matmuls in different 32-row bands)
    flat software pipeline over chunks (h, ci) and key-tile groups, using two
    alternating PSUM staging pools (4 + 3 banks) shared with the projection
    stream, PV lagging exp by two groups so activations run back-to-back:
        simT[kt] = krep_h[band, kt].T @ qrep_h[band, ci]   -> PSUM
        probs = exp(simT)     (ScalarE, PSUM->SBUF, bf16)
        pv += [v_h|1].T @ probs  (accumulate all 32 kt)    -> [33, 512]
    rows 0..31 = unnormalized out, row 32 = softmax denominator;
    outh[ci][32h:] = pv[0:32] * bcast(1/pv[32]) (recip + DRAM-bounce DMA)
    out[ci] = W_o @ outh[ci] + b_out  -> DMA out
"""

import numpy as np
import ml_dtypes

import concourse.bass as bass
import concourse.mybir as mybir
import concourse.tile as tile
from concourse import bacc
from concourse.bass import ts, ds
from concourse.bass_utils import run_bass_kernel_spmd

HEADS = 4
D = 32
HID = 128
C = 256
N = 4096
NQ = 2048
SCALE = D ** -0.5
NCORES = 8

F32 = mybir.dt.float32
F32R = mybir.dt.float32r
BF16 = mybir.dt.bfloat16
EXP = mybir.ActivationFunctionType.Exp

NKT = N // 128  # 32 key tiles per chunk
NCH = NQ // 512  # 4 query chunks
PVLAG = 2  # PV trails its exp by this many staging groups


def build_nc():
    nc = bacc.Bacc("TRN2")

    xb = nc.declare_dram_parameter("xb", [C, N], BF16, isOutput=False)
    xq = nc.declare_dram_parameter("xq", [C, NQ], BF16, isOutput=False)
    wqrT = nc.declare_dram_parameter("wqrT", [C, HEADS * HID], BF16, isOutput=False)
    wkrT = nc.declare_dram_parameter("wkrT", [C, HEADS * HID], BF16, isOutput=False)
    wvT = nc.declare_dram_parameter("wvT", [C, HID], BF16, isOutput=False)
    woT = nc.declare_dram_parameter("woT", [HID, C], F32R, isOutput=False)
    bout = nc.declare_dram_parameter("bout", [C, 1], F32, isOutput=False)
    out = nc.declare_dram_parameter("out", [C, NQ], F32, isOutput=True)

    with tile.TileContext(nc) as tc:
        with (
            nc.allow_low_precision(reason="bf16/fp32r attention core"),
            tc.tile_pool(name="persist", bufs=1) as persist,
            tc.tile_pool(name="wts", bufs=1) as wts,
            tc.tile_pool(name="dram", bufs=2, space="DRAM") as dram_pool,
        ):
            # ---- persistent SBUF tensors ----
            x_sb = [
                [
                    persist.tile([128, N // 4], BF16, tag=f"x{i}{j}", name=f"x{i}{j}")
                    for j in range(4)
                ]
                for i in range(2)
            ]
            xq_sb = [
                [
                    persist.tile([128, NQ // 2], BF16, tag=f"xq{i}{j}", name=f"xq{i}{j}")
                    for j in range(2)
                ]
                for i in range(2)
            ]
            krep = [
                persist.tile([128, N], BF16, tag=f"krep{h}", name=f"krep{h}")
                for h in range(HEADS)
            ]
            qrep = [
                persist.tile([128, NQ], BF16, tag=f"qrep{h}", name=f"qrep{h}")
                for h in range(HEADS)
            ]
            # per key-tile: 4 head blocks of [v_h (32) | ones (1)]
            vT_sb = persist.tile([128, NKT * 132], BF16, tag="vT")

            wqr_sb = [
                wts.tile([128, HEADS * HID], BF16, tag=f"wqr{i}", name=f"wqr{i}")
                for i in range(2)
            ]
            wkr_sb = [
                wts.tile([128, HEADS * HID], BF16, tag=f"wkr{i}", name=f"wkr{i}")
                for i in range(2)
            ]
            wv_sb = [
                wts.tile([128, HID], BF16, tag=f"wv{i}", name=f"wv{i}")
                for i in range(2)
            ]
            wo_sb = wts.tile([HID, C], F32R, tag="wo")
            bo_sb = [
                wts.tile([128, 1], F32, tag=f"bo{i}", name=f"bo{i}")
                for i in range(2)
            ]
            ones_sb = wts.tile([1, D], F32, tag="ones")

            # ---- DMA inputs: weights on sync sequencer, x/xq on gpsimd
            # (parallel issue; ~0.6us sequencer cost per dma_start)
            for i in range(2):
                nc.sync.dma_start(out=wqr_sb[i][:], in_=wqrT[ds(i * 128, 128), :])
                nc.sync.dma_start(out=wkr_sb[i][:], in_=wkrT[ds(i * 128, 128), :])
                nc.sync.dma_start(out=wv_sb[i][:], in_=wvT[ds(i * 128, 128), :])
                nc.sync.dma_start(out=bo_sb[i][:], in_=bout[ds(i * 128, 128), :])
            nc.sync.dma_start(out=wo_sb[:], in_=woT[:, :])
            for i in range(2):
                for j in range(4):
                    nc.gpsimd.dma_start(
                        out=x_sb[i][j][:],
                        in_=xb[ds(i * 128, 128), ts(j, N // 4)],
                    )
                for j in range(2):
                    nc.gpsimd.dma_start(
                        out=xq_sb[i][j][:],
                        in_=xq[ds(i * 128, 128), ts(j, NQ // 2)],
                    )
            nc.vector.memset(vT_sb[:], 1.0)
            nc.vector.memset(ones_sb[:], 1.0)

            with (
                tc.tile_pool(name="qkA", bufs=1, space="PSUM") as qkA,
                tc.tile_pool(name="qkB", bufs=1, space="PSUM") as qkB,
                tc.tile_pool(name="pvp", bufs=1, space="PSUM") as pvp,
                tc.tile_pool(name="probs", bufs=6) as probs_pool,
                tc.tile_pool(name="norm", bufs=3) as norm_pool,
                tc.tile_pool(name="osb", bufs=2) as osb,
            ):
                # staging slots rotate globally between the two pools;
                # projection tiles share the same rotation (no extra banks)
                _ptog = [0]

                def x_ap(ct, c0, length):
                    t_idx = c0 // (N // 4)
                    return x_sb[ct][t_idx][:, ds(c0 % (N // 4), length)]

                def xq_ap(ct, c0, length):
                    t_idx = c0 // (NQ // 2)
                    return xq_sb[ct][t_idx][:, ds(c0 % (NQ // 2), length)]

                def next_pool():
                    pool = qkA if _ptog[0] == 0 else qkB
                    _ptog[0] ^= 1
                    return pool

                def proj_tile(cols):
                    pool = next_pool()
                    t = pool.tile(
                        [128, (4 if pool is qkA else 3) * 512],
                        F32,
                        tag="qk",
                        name="ps",
                    )
                    return t[:, 0:cols]

                def emit_vt4(kt0):
                    # four key tiles' vT in one staging slot, one strided copy
                    ps = proj_tile(4 * HID)
                    for t in range(4):
                        for ct in range(2):
                            nc.tensor.matmul(
                                ps[:, ts(t, HID)],
                                x_ap(ct, (kt0 + t) * 128, 128),
                                wv_sb[ct][:],
                                start=(ct == 0),
                                stop=(ct == 1),
                            )
                    dst = vT_sb[:, ds(kt0 * 132, 4 * 132)].rearrange(
                        "p (t h w) -> p t h w", t=4, w=33
                    )[:, :, :, 0:32]
                    src = ps.rearrange("p (t w) -> p t w", t=4).rearrange(
                        "p t (h w) -> p t h w", w=32
                    )
                    nc.vector.tensor_copy(dst, src)

                def emit_k(h, j):
                    ps = proj_tile(512)
                    for ct in range(2):
                        nc.tensor.matmul(
                            ps[:],
                            wkr_sb[ct][:, ts(h, HID)],
                            x_ap(ct, j * 512, 512),
                            start=(ct == 0),
                            stop=(ct == 1),
                        )
                    nc.vector.tensor_copy(krep[h][:, ts(j, 512)], ps[:])

                def emit_q(h, j):
                    ps = proj_tile(512)
                    for ct in range(2):
                        nc.tensor.matmul(
                            ps[:],
                            wqr_sb[ct][:, ts(h, HID)],
                            xq_ap(ct, j * 512, 512),
                            start=(ct == 0),
                            stop=(ct == 1),
                        )
                    nc.vector.tensor_copy(qrep[h][:, ts(j, 512)], ps[:])

                outh = [
                    osb.tile([HID, 512], F32R, tag=f"outh{c}", name=f"outh{c}")
                    for c in range(NCH)
                ]

                def emit_norm(h, ci, pv):
                    # rows 0..31 / row 32
                    pvs = norm_pool.tile([33, 512], F32, tag="pvs", name="pvs")
                    nc.vector.tensor_copy(pvs[:], pv[0:33, :])
                    rec = norm_pool.tile([1, 512], F32, tag="rec", name="rec")
                    nc.vector.reciprocal(rec[:], pvs[32:33, :])
                    # broadcast 1/denom to 32 partitions via ones-matmul
                    # (f32, C=1 -- cheap; lands in a staging rotation slot)
                    bct = next_pool()
                    bc = bct.tile(
                        [128, (4 if bct is qkA else 3) * 512],
                        F32,
                        tag="qk",
                        name="bc",
                    )
                    nc.tensor.matmul(
                        bc[0:D, 0:512], ones_sb[:], rec[:], start=True, stop=True
                    )
                    nc.vector.tensor_mul(
                        outh[ci][ds(32 * h, 32), :], pvs[0:32, :], bc[0:D, 0:512]
                    )

                pending = []

                def emit_outproj(ci):
                    for ot in range(2):
                        op = pvp.tile([128, 512], F32, tag="pv", name="op")
                        nc.tensor.matmul(
                            op[:],
                            wo_sb[:, ts(ot, 128)],
                            outh[ci][:],
                            start=True,
                            stop=True,
                        )
                        ob = osb.tile([128, 512], F32, tag="ob", name="ob")
                        nc.vector.tensor_scalar_add(ob[:], op[:], bo_sb[ot][:])
                        nc.sync.dma_start(
                            out=out[ds(ot * 128, 128), ts(ci, 512)], in_=ob[:]
                        )

                def pop_pv():
                    probs, kt0, gsz, h, ci, pv = pending.pop(0)
                    for j in range(gsz):
                        nc.tensor.matmul(
                            pv[0:33, :],
                            vT_sb[:, ds((kt0 + j) * 132 + 33 * h, 33)],
                            probs[:, ts(j, 512)],
                            start=(kt0 + j == 0),
                            stop=(kt0 + j == NKT - 1),
                        )
                    if kt0 + gsz == NKT:
                        emit_norm(h, ci, pv)
                        if h == HEADS - 1:
                            emit_outproj(ci)

                # prologue: first projections
                emit_k(0, 0)
                emit_k(0, 1)
                emit_vt4(0)
                emit_q(0, 0)

                for h in range(HEADS):
                    for ci in range(NCH):
                        pv = pvp.tile([128, 512], F32, tag="pv", name="pv")
                        kt = 0
                        g = -1
                        while kt < NKT:
                            g += 1
                            pool = next_pool()
                            gsz = min(4 if pool is qkA else 3, NKT - kt)
                            qk = pool.tile(
                                [128, gsz * 512], F32, tag="qk", name="qkg"
                            )
                            for j in range(gsz):
                                band = (kt + j) % 4
                                nc.tensor.matmul(
                                    qk[:, ts(j, 512)],
                                    krep[h][ds(32 * band, 32), ts(kt + j, 128)],
                                    qrep[h][ds(32 * band, 32), ts(ci, 512)],
                                    start=True,
                                    stop=True,
                                    tile_position=(32 * band, 0),
                                )
                            probs = probs_pool.tile(
                                [128, gsz * 512], BF16, tag="pr", name="pr"
                            )
                            nc.scalar.activation(probs[:], qk[:], EXP)
                            pending.append((probs, kt, gsz, h, ci, pv))
                            if len(pending) > PVLAG:
                                pop_pv()
                            kt += gsz
                            # feed upcoming projections into PE idle slots
                            if ci == 0 and h == 0 and g < 7:
                                if g < 6:
                                    emit_k(h, g + 2)
                                if 4 * g + 4 < NKT:
                                    emit_vt4(4 * g + 4)
                            if ci == 0 and h > 0 and 2 <= g < 6:
                                emit_k(h, g + 2)
                            if g == 1 and ci < NCH - 1:
                                emit_q(h, ci + 1)
                            if ci == NCH - 1 and h < HEADS - 1 and 2 <= g < 6:
                                emit_k(h + 1, g - 2)
                                if g == 2:
                                    emit_q(h + 1, 0)
                while pending:
                    pop_pv()

    nc.finalize()
    return nc


_NC_CACHE = None


def make_in_maps(x, w_qkv, w_out, b_out):
    bf16 = ml_dtypes.bfloat16
    x = np.ascontiguousarray(np.asarray(x, dtype=np.float32)).reshape(4, C, N)
    w_qkv = np.asarray(w_qkv, dtype=np.float32)
    w_out = np.asarray(w_out, dtype=np.float32)
    b_out = np.asarray(b_out, dtype=np.float32)

    wqT = (w_qkv[0:HID] * SCALE).T                              # [256, 128]
    wkT = w_qkv[HID:2 * HID].T                                  # [256, 128]
    # per-head projection weights, head block replicated 4x along columns
    wqrT = np.ascontiguousarray(
        np.concatenate(
            [np.tile(wqT[:, 32 * h:32 * (h + 1)], (1, 4)) for h in range(HEADS)],
            axis=1,
        )
    ).astype(bf16)
    wkrT = np.ascontiguousarray(
        np.concatenate(
            [np.tile(wkT[:, 32 * h:32 * (h + 1)], (1, 4)) for h in range(HEADS)],
            axis=1,
        )
    ).astype(bf16)
    wvT = np.ascontiguousarray(w_qkv[2 * HID:3 * HID].T).astype(bf16)
    woT = np.ascontiguousarray(w_out.T)                         # [128, 256]
    boutc = np.ascontiguousarray(b_out.reshape(C, 1))
    xbf = x.astype(bf16)

    in_maps = []
    for core in range(NCORES):
        b, half = divmod(core, 2)
        in_maps.append(
            {
                "xb": xbf[b],
                "xq": np.ascontiguousarray(xbf[b][:, half * NQ:(half + 1) * NQ]),
                "wqrT": wqrT,
                "wkrT": wkrT,
                "wvT": wvT,
                "woT": woT,
                "bout": boutc,
            }
        )
    return in_maps


def kernel(x, w_qkv, w_out, b_out):
    global _NC_CACHE
    if _NC_CACHE is None:
        _NC_CACHE = build_nc()
    nc = _NC_CACHE
    in_maps = make_in_maps(x, w_qkv, w_out, b_out)
    res = run_bass_kernel_spmd(nc, in_maps, core_ids=list(range(NCORES)))
    out = np.empty((4, C, N), dtype=np.float32)
    for core in range(NCORES):
        b, half = divmod(core, 2)
        out[b][:, half * NQ:(half + 1) * NQ] = res.results[core]["out"]
    return out.reshape(4, C, 64, 64)


# revision 19
# speedup vs baseline: 1.0802x; 1.0802x over previous
"""Trainium2 Bass kernel for nn_Attention_21715354649378.

Reference computation (per batch b of 4):
    qkv = w_qkv @ x        x: [256, 4096(=64x64)]   w_qkv: [384, 256]
    q,k,v: [4 heads, 32, 4096];  q *= 32**-0.5
    sim_h = q_h^T k_h   [4096, 4096];  attn = softmax(sim, axis=-1)
    out_h = attn @ v_h^T    -> [4096, 32]
    out = w_out @ concat_heads + b_out   [256, 4096]

Sharding: 8 cores = 4 batches x 2 query-halves. Each core computes K/V for
its full batch plus attention + output projection for its half of the query
pixels. Outputs are disjoint slices -> no collectives.

Device algorithm per core (keys-in-partition layout, no max-subtraction --
sim values are O(6) so exp is safe in f32):
    vT = x^T W_v^T          per 128-key tile, 4 head blocks [v_h (32) | 1]
    krep_h = repl4(W_k,h) x   [128 = 4 copies of k_h(32d), 4096]  bf16
    qrep_h = repl4(s W_q,h) xq [128, 2048] bf16
      (replication lets QK use PE row-group kt%4: fast-weight-loads overlap
       and up to 4 concurrent matmuls in different 32-row bands)
    flat software pipeline over chunks (h, ci) and key-tile groups, using two
    alternating PSUM staging pools (4 + 3 banks) shared with the projection
    stream, PV lagging exp by two groups so activations run back-to-back:
        simT[kt] = krep_h[band, kt].T @ qrep_h[band, ci]   -> PSUM
        probs = exp(simT)     (ScalarE, PSUM->SBUF, bf16)
        pv += [v_h|1].T @ probs  (accumulate all 32 kt)    -> [33, 512]
    rows 0..31 = unnormalized out, row 32 = softmax denominator;
    outh[ci][32h:] = pv[0:32] * bcast(1/pv[32]) (recip + DRAM-bounce DMA)
    out[ci] = W_o @ outh[ci] + b_out  -> DMA out
"""

import numpy as np
import ml_dtypes

import concourse.bass as bass
import concourse.mybir as mybir
import concourse.tile as tile
from concourse import bacc
from concourse.bass import ts, ds
from concourse.bass_utils import run_bass_kernel_spmd

HEADS = 4
D = 32
HID = 128
C = 256
N = 4096
NQ = 2048
SCALE = D ** -0.5
NCORES = 8

F32 = mybir.dt.float32
F32R = mybir.dt.float32r
BF16 = mybir.dt.bfloat16
EXP = mybir.ActivationFunctionType.Exp

NKT = N // 128  # 32 key tiles per chunk
NCH = NQ // 512  # 4 query chunks
PVLAG = 2  # PV trails its exp by this many staging groups


def build_nc():
    nc = bacc.Bacc("TRN2")

    xb = nc.declare_dram_parameter("xb", [C, N], BF16, isOutput=False)
    xq = nc.declare_dram_parameter("xq", [C, NQ], BF16, isOutput=False)
    wqrT = nc.declare_dram_parameter("wqrT", [C, HEADS * HID], BF16, isOutput=False)
    wkrT = nc.declare_dram_parameter("wkrT", [C, HEADS * HID], BF16, isOutput=False)
    wvT = nc.declare_dram_parameter("wvT", [C, HID], BF16, isOutput=False)
    woT = nc.declare_dram_parameter("woT", [HID, C], F32R, isOutput=False)
    bout = nc.declare_dram_parameter("bout", [C, 1], F32, isOutput=False)
    out = nc.declare_dram_parameter("out", [C, NQ], F32, isOutput=True)

    with tile.TileContext(nc) as tc:
        with (
            nc.allow_low_precision(reason="bf16/fp32r attention core"),
            tc.tile_pool(name="persist", bufs=1) as persist,
            tc.tile_pool(name="wts", bufs=1) as wts,
            tc.tile_pool(name="dram", bufs=2, space="DRAM") as dram_pool,
        ):
            # ---- persistent SBUF tensors ----
            x_sb = [
                [
                    persist.tile([128, N // 4], BF16, tag=f"x{i}{j}", name=f"x{i}{j}")
                    for j in range(4)
                ]
                for i in range(2)
            ]
            xq_sb = [
                [
                    persist.tile([128, NQ // 2], BF16, tag=f"xq{i}{j}", name=f"xq{i}{j}")
                    for j in range(2)
                ]
                for i in range(2)
            ]
            krep = [
                persist.tile([128, N], BF16, tag=f"krep{h}", name=f"krep{h}")
                for h in range(HEADS)
            ]
            qrep = [
                persist.tile([128, NQ], BF16, tag=f"qrep{h}", name=f"qrep{h}")
                for h in range(HEADS)
            ]
            # per key-tile: 4 head blocks of [v_h (32) | ones (1)]
            vT_sb = persist.tile([128, NKT * 132], BF16, tag="vT")

            wqr_sb = [
                wts.tile([128, HEADS * HID], BF16, tag=f"wqr{i}", name=f"wqr{i}")
                for i in range(2)
            ]
            wkr_sb = [
                wts.tile([128, HEADS * HID], BF16, tag=f"wkr{i}", name=f"wkr{i}")
                for i in range(2)
            ]
            wv_sb = [
                wts.tile([128, HID], BF16, tag=f"wv{i}", name=f"wv{i}")
                for i in range(2)
            ]
            wo_sb = wts.tile([HID, C], F32R, tag="wo")
            bo_sb = [
                wts.tile([128, 1], F32, tag=f"bo{i}", name=f"bo{i}")
                for i in range(2)
            ]
            ones_sb = wts.tile([1, D], F32, tag="ones")

            # ---- DMA inputs: weights on sync sequencer, x/xq on gpsimd
            # (parallel issue; ~0.6us sequencer cost per dma_start)
            for i in range(2):
                nc.sync.dma_start(out=wqr_sb[i][:], in_=wqrT[ds(i * 128, 128), :])
                nc.sync.dma_start(out=wkr_sb[i][:], in_=wkrT[ds(i * 128, 128), :])
                nc.sync.dma_start(out=wv_sb[i][:], in_=wvT[ds(i * 128, 128), :])
                nc.sync.dma_start(out=bo_sb[i][:], in_=bout[ds(i * 128, 128), :])
            nc.sync.dma_start(out=wo_sb[:], in_=woT[:, :])
            for i in range(2):
                for j in range(4):
                    nc.gpsimd.dma_start(
                        out=x_sb[i][j][:],
                        in_=xb[ds(i * 128, 128), ts(j, N // 4)],
                    )
                for j in range(2):
                    nc.gpsimd.dma_start(
                        out=xq_sb[i][j][:],
                        in_=xq[ds(i * 128, 128), ts(j, NQ // 2)],
                    )
            nc.vector.memset(vT_sb[:], 1.0)
            nc.vector.memset(ones_sb[:], 1.0)

            with (
                tc.tile_pool(name="qkA", bufs=1, space="PSUM") as qkA,
                tc.tile_pool(name="qkB", bufs=1, space="PSUM") as qkB,
                tc.tile_pool(name="pvp", bufs=1, space="PSUM") as pvp,
                tc.tile_pool(name="probs", bufs=6) as probs_pool,
                tc.tile_pool(name="norm", bufs=3) as norm_pool,
                tc.tile_pool(name="osb", bufs=2) as osb,
            ):
                # staging slots rotate globally between the two pools;
                # projection tiles share the same rotation (no extra banks)
                _ptog = [0]

                def x_ap(ct, c0, length):
                    t_idx = c0 // (N // 4)
                    return x_sb[ct][t_idx][:, ds(c0 % (N // 4), length)]

                def xq_ap(ct, c0, length):
                    t_idx = c0 // (NQ // 2)
                    return xq_sb[ct][t_idx][:, ds(c0 % (NQ // 2), length)]

                def next_pool():
                    pool = qkA if _ptog[0] == 0 else qkB
                    _ptog[0] ^= 1
                    return pool

                def proj_tile(cols):
                    pool = next_pool()
                    t = pool.tile(
                        [128, (4 if pool is qkA else 3) * 512],
                        F32,
                        tag="qk",
                        name="ps",
                    )
                    return t[:, 0:cols]

                def emit_vt4(kt0):
                    # four key tiles' vT in one staging slot, one strided copy
                    ps = proj_tile(4 * HID)
                    for t in range(4):
                        for ct in range(2):
                            nc.tensor.matmul(
                                ps[:, ts(t, HID)],
                                x_ap(ct, (kt0 + t) * 128, 128),
                                wv_sb[ct][:],
                                start=(ct == 0),
                                stop=(ct == 1),
                            )
                    dst = vT_sb[:, ds(kt0 * 132, 4 * 132)].rearrange(
                        "p (t h w) -> p t h w", t=4, w=33
                    )[:, :, :, 0:32]
                    src = ps.rearrange("p (t w) -> p t w", t=4).rearrange(
                        "p t (h w) -> p t h w", w=32
                    )
                    nc.vector.tensor_copy(dst, src)

                def emit_k(h, j):
                    ps = proj_tile(512)
                    for ct in range(2):
                        nc.tensor.matmul(
                            ps[:],
                            wkr_sb[ct][:, ts(h, HID)],
                            x_ap(ct, j * 512, 512),
                            start=(ct == 0),
                            stop=(ct == 1),
                        )
                    nc.vector.tensor_copy(krep[h][:, ts(j, 512)], ps[:])

                def emit_q(h, j):
                    ps = proj_tile(512)
                    for ct in range(2):
                        nc.tensor.matmul(
                            ps[:],
                            wqr_sb[ct][:, ts(h, HID)],
                            xq_ap(ct, j * 512, 512),
                            start=(ct == 0),
                            stop=(ct == 1),
                        )
                    nc.vector.tensor_copy(qrep[h][:, ts(j, 512)], ps[:])

                outh = [
                    osb.tile([HID, 512], F32R, tag=f"outh{c}", name=f"outh{c}")
                    for c in range(NCH)
                ]

                def emit_norm(h, ci, pv):
                    # rows 0..31 / row 32
                    pvs = norm_pool.tile([33, 512], F32, tag="pvs", name="pvs")
                    nc.vector.tensor_copy(pvs[:], pv[0:33, :])
                    rec = norm_pool.tile([1, 512], F32, tag="rec", name="rec")
                    nc.vector.reciprocal(rec[:], pvs[32:33, :])
                    # broadcast 1/denom to 32 partitions via DRAM bounce
                    rdr = dram_pool.tile([1, 512], F32, tag="rdr", name="rdr")
                    nc.scalar.dma_start(out=rdr[:], in_=rec[:])
                    bc = norm_pool.tile([D, 512], F32, tag="bc", name="bc")
                    nc.scalar.dma_start(
                        out=bc[:],
                        in_=bass.AP(
                            tensor=rdr.tensor,
                            offset=rdr.offset,
                            ap=[[0, D]] + [list(a) for a in rdr.ap[1:]],
                        ),
                    )
                    nc.vector.tensor_mul(
                        outh[ci][ds(32 * h, 32), :], pvs[0:32, :], bc[:]
                    )

                pending = []

                def emit_outproj(ci):
                    for ot in range(2):
                        op = pvp.tile([128, 512], F32, tag="pv", name="op")
                        nc.tensor.matmul(
                            op[:],
                            wo_sb[:, ts(ot, 128)],
                            outh[ci][:],
                            start=True,
                            stop=True,
                        )
                        ob = osb.tile([128, 512], F32, tag="ob", name="ob")
                        nc.vector.tensor_scalar_add(ob[:], op[:], bo_sb[ot][:])
                        nc.sync.dma_start(
                            out=out[ds(ot * 128, 128), ts(ci, 512)], in_=ob[:]
                        )

                def pop_pv():
                    probs, kt0, gsz, h, ci, pv = pending.pop(0)
                    for j in range(gsz):
                        nc.tensor.matmul(
                            pv[0:33, :],
                            vT_sb[:, ds((kt0 + j) * 132 + 33 * h, 33)],
                            probs[:, ts(j, 512)],
                            start=(kt0 + j == 0),
                            stop=(kt0 + j == NKT - 1),
                        )
                    if kt0 + gsz == NKT:
                        emit_norm(h, ci, pv)
                        if h == HEADS - 1:
                            emit_outproj(ci)

                # prologue: first projections
                emit_k(0, 0)
                emit_k(0, 1)
                emit_vt4(0)
                emit_q(0, 0)

                for h in range(HEADS):
                    for ci in range(NCH):
                        pv = pvp.tile([128, 512], F32, tag="pv", name="pv")
                        kt = 0
                        g = -1
                        while kt < NKT:
                            g += 1
                            pool = next_pool()
                            gsz = min(4 if pool is qkA else 3, NKT - kt)
                            qk = pool.tile(
                                [128, gsz * 512], F32, tag="qk", name="qkg"
                            )
                            for j in range(gsz):
                                band = (kt + j) % 4
                                nc.tensor.matmul(
                                    qk[:, ts(j, 512)],
                                    krep[h][ds(32 * band, 32), ts(kt + j, 128)],
                                    qrep[h][ds(32 * band, 32), ts(ci, 512)],
                                    start=True,
                                    stop=True,
                                    tile_position=(32 * band, 0),
                                )
                            probs = probs_pool.tile(
                                [128, gsz * 512], BF16, tag="pr", name="pr"
                            )
                            nc.scalar.activation(probs[:], qk[:], EXP)
                            pending.append((probs, kt, gsz, h, ci, pv))
                            if len(pending) > PVLAG:
                                pop_pv()
                            kt += gsz
                            # feed upcoming projections into PE idle slots
                            if ci == 0 and h == 0 and g < 7:
                                if g < 6:
                                    emit_k(h, g + 2)
                                if 4 * g + 4 < NKT:
                                    emit_vt4(4 * g + 4)
                            if ci == 0 and h > 0 and 2 <= g < 6:
                                emit_k(h, g + 2)
                            if g == 1 and ci < NCH - 1:
                                emit_q(h, ci + 1)
                            if ci == NCH - 1 and h < HEADS - 1 and 2 <= g < 6:
                                emit_k(h + 1, g - 2)
                                if g == 2:
                                    emit_q(h + 1, 0)
                while pending:
                    pop_pv()

    nc.finalize()
    return nc


_NC_CACHE = None


def make_in_maps(x, w_qkv, w_out, b_out):
    bf16 = ml_dtypes.bfloat16
    x = np.ascontiguousarray(np.asarray(x, dtype=np.float32)).reshape(4, C, N)
    w_qkv = np.asarray(w_qkv, dtype=np.float32)
    w_out = np.asarray(w_out, dtype=np.float32)
    b_out = np.asarray(b_out, dtype=np.float32)

    wqT = (w_qkv[0:HID] * SCALE).T                              # [256, 128]
    wkT = w_qkv[HID:2 * HID].T                                  # [256, 128]
    # per-head projection weights, head block replicated 4x along columns
    wqrT = np.ascontiguousarray(
        np.concatenate(
            [np.tile(wqT[:, 32 * h:32 * (h + 1)], (1, 4)) for h in range(HEADS)],
            axis=1,
        )
    ).astype(bf16)
    wkrT = np.ascontiguousarray(
        np.concatenate(
            [np.tile(wkT[:, 32 * h:32 * (h + 1)], (1, 4)) for h in range(HEADS)],
            axis=1,
        )
    ).astype(bf16)
    wvT = np.ascontiguousarray(w_qkv[2 * HID:3 * HID].T).astype(bf16)
    woT = np.ascontiguousarray(w_out.T)                         # [128, 256]
    boutc = np.ascontiguousarray(b_out.reshape(C, 1))
    xbf = x.astype(bf16)

    in_maps = []
    for core in range(NCORES):
        b, half = divmod(core, 2)
        in_maps.append(
            {
                "xb": xbf[b],
                "xq": np.ascontiguousarray(xbf[b][:, half * NQ:(half + 1) * NQ]),
                "wqrT": wqrT,
                "wkrT": wkrT,
                "wvT": wvT,
                "woT": woT,
                "bout": boutc,
            }
        )
    return in_maps


def kernel(x, w_qkv, w_out, b_out):
    global _NC_CACHE
    if _NC_CACHE is None:
        _NC_CACHE = build_nc()
    nc = _NC_CACHE
    in_maps = make_in_maps(x, w_qkv, w_out, b_out)
    res = run_bass_kernel_spmd(nc, in_maps, core_ids=list(range(NCORES)))
    out = np.empty((4, C, N), dtype=np.float32)
    for core in range(NCORES):
        b, half = divmod(core, 2)
        out[b][:, half * NQ:(half + 1) * NQ] = res.results[core]["out"]
    return out.reshape(4, C, 64, 64)


# revision 21
# speedup vs baseline: 1.1748x; 1.0876x over previous
"""Trainium2 Bass kernel for nn_Attention_21715354649378.

Reference computation (per batch b of 4):
    qkv = w_qkv @ x        x: [256, 4096(=64x64)]   w_qkv: [384, 256]
    q,k,v: [4 heads, 32, 4096];  q *= 32**-0.5
    sim_h = q_h^T k_h   [4096, 4096];  attn = softmax(sim, axis=-1)
    out_h = attn @ v_h^T    -> [4096, 32]
    out = w_out @ concat_heads + b_out   [256, 4096]

Sharding: 8 cores = 4 batches x 2 query-halves. Each core computes K/V for
its full batch plus attention + output projection for its half of the query
pixels. Outputs are disjoint slices -> no collectives.

Device algorithm per core (keys-in-partition layout, no max-subtraction --
sim values are O(6) so exp is safe in f32):
    vT = x^T W_v^T          per 128-key tile, 4 head blocks [v_h (32) | 1]
    krep_h = repl4(W_k,h) x   [128 = 4 copies of k_h(32d), 4096]  bf16
    qrep_h = repl4(s W_q,h) xq [128, 2048] bf16
      (replication lets QK use PE row-group kt%4: fast-weight-loads overlap
       and up to 4 concurrent matmuls in different 32-row bands)
    flat software pipeline over chunks (h, ci) and key-tile groups, using two
    alternating PSUM staging pools (4 + 3 banks) shared with the projection
    stream, PV lagging exp by two groups so activations run back-to-back:
        simT[kt] = krep_h[band, kt].T @ qrep_h[band, ci]   -> PSUM
        probs = exp(simT)     (ScalarE, PSUM->SBUF, bf16)
        pv += [v_h|1].T @ probs  (accumulate all 32 kt)    -> [33, 512]
    rows 0..31 = unnormalized out, row 32 = softmax denominator;
    outh[ci][32h:] = pv[0:32] * bcast(1/pv[32]) (recip + DRAM-bounce DMA)
    out[ci] = W_o @ outh[ci] + b_out  -> DMA out
"""

import numpy as np
import ml_dtypes

import concourse.bass as bass
import concourse.mybir as mybir
import concourse.tile as tile
from concourse import bacc
from concourse.bass import ts, ds
from concourse.bass_utils import run_bass_kernel_spmd

HEADS = 4
D = 32
HID = 128
C = 256
N = 4096
NQ = 2048
SCALE = D ** -0.5
NCORES = 8

F32 = mybir.dt.float32
F32R = mybir.dt.float32r
BF16 = mybir.dt.bfloat16
EXP = mybir.ActivationFunctionType.Exp

NKT = N // 128  # 32 key tiles per chunk
NCH = NQ // 512  # 4 query chunks
PVLAG = 2  # PV trails its exp by this many staging groups


def build_nc():
    nc = bacc.Bacc("TRN2")

    xb = nc.declare_dram_parameter("xb", [C, N], BF16, isOutput=False)
    xq = nc.declare_dram_parameter("xq", [C, NQ], BF16, isOutput=False)
    wqrT = nc.declare_dram_parameter("wqrT", [C, HEADS * HID], BF16, isOutput=False)
    wkrT = nc.declare_dram_parameter("wkrT", [C, HEADS * HID], BF16, isOutput=False)
    wvT = nc.declare_dram_parameter("wvT", [C, HID], BF16, isOutput=False)
    woT = nc.declare_dram_parameter("woT", [HID, C], F32R, isOutput=False)
    bout = nc.declare_dram_parameter("bout", [C, 1], F32, isOutput=False)
    out = nc.declare_dram_parameter("out", [C, NQ], F32, isOutput=True)

    with tile.TileContext(nc) as tc:
        with (
            nc.allow_low_precision(reason="bf16/fp32r attention core"),
            tc.tile_pool(name="persist", bufs=1) as persist,
            tc.tile_pool(name="wts", bufs=1) as wts,
            tc.tile_pool(name="dram", bufs=2, space="DRAM") as dram_pool,
        ):
            # ---- persistent SBUF tensors ----
            x_sb = [
                [
                    persist.tile([128, N // 4], BF16, tag=f"x{i}{j}", name=f"x{i}{j}")
                    for j in range(4)
                ]
                for i in range(2)
            ]
            xq_sb = [
                [
                    persist.tile([128, NQ // 2], BF16, tag=f"xq{i}{j}", name=f"xq{i}{j}")
                    for j in range(2)
                ]
                for i in range(2)
            ]
            krep = [
                persist.tile([128, N], BF16, tag=f"krep{h}", name=f"krep{h}")
                for h in range(HEADS)
            ]
            qrep = [
                persist.tile([128, NQ], BF16, tag=f"qrep{h}", name=f"qrep{h}")
                for h in range(HEADS)
            ]
            # per key-tile: 4 head blocks of [v_h (32) | ones (1)]
            vT_sb = persist.tile([128, NKT * 132], BF16, tag="vT")

            wqr_sb = [
                wts.tile([128, HEADS * HID], BF16, tag=f"wqr{i}", name=f"wqr{i}")
                for i in range(2)
            ]
            wkr_sb = [
                wts.tile([128, HEADS * HID], BF16, tag=f"wkr{i}", name=f"wkr{i}")
                for i in range(2)
            ]
            wv_sb = [
                wts.tile([128, HID], BF16, tag=f"wv{i}", name=f"wv{i}")
                for i in range(2)
            ]
            wo_sb = wts.tile([HID, C], F32R, tag="wo")
            bo_sb = [
                wts.tile([128, 1], F32, tag=f"bo{i}", name=f"bo{i}")
                for i in range(2)
            ]
            ones_sb = wts.tile([1, D], F32, tag="ones")

            # ---- DMA inputs: weights on sync sequencer, x/xq on gpsimd
            # (parallel issue; ~0.6us sequencer cost per dma_start)
            for i in range(2):
                nc.sync.dma_start(out=wqr_sb[i][:], in_=wqrT[ds(i * 128, 128), :])
                nc.sync.dma_start(out=wkr_sb[i][:], in_=wkrT[ds(i * 128, 128), :])
                nc.sync.dma_start(out=wv_sb[i][:], in_=wvT[ds(i * 128, 128), :])
                nc.sync.dma_start(out=bo_sb[i][:], in_=bout[ds(i * 128, 128), :])
            nc.sync.dma_start(out=wo_sb[:], in_=woT[:, :])
            for i in range(2):
                for j in range(4):
                    nc.gpsimd.dma_start(
                        out=x_sb[i][j][:],
                        in_=xb[ds(i * 128, 128), ts(j, N // 4)],
                    )
                for j in range(2):
                    nc.gpsimd.dma_start(
                        out=xq_sb[i][j][:],
                        in_=xq[ds(i * 128, 128), ts(j, NQ // 2)],
                    )
            nc.vector.memset(vT_sb[:], 1.0)
            nc.vector.memset(ones_sb[:], 1.0)

            with (
                tc.tile_pool(name="qkA", bufs=1, space="PSUM") as qkA,
                tc.tile_pool(name="qkB", bufs=1, space="PSUM") as qkB,
                tc.tile_pool(name="pvp", bufs=1, space="PSUM") as pvp,
                tc.tile_pool(name="probs", bufs=6) as probs_pool,
                tc.tile_pool(name="norm", bufs=3) as norm_pool,
                tc.tile_pool(name="osb", bufs=2) as osb,
            ):
                # staging slots rotate globally between the two pools;
                # projection tiles share the same rotation (no extra banks)
                _ptog = [0]

                def x_ap(ct, c0, length):
                    t_idx = c0 // (N // 4)
                    return x_sb[ct][t_idx][:, ds(c0 % (N // 4), length)]

                def xq_ap(ct, c0, length):
                    t_idx = c0 // (NQ // 2)
                    return xq_sb[ct][t_idx][:, ds(c0 % (NQ // 2), length)]

                def next_pool():
                    pool = qkA if _ptog[0] == 0 else qkB
                    _ptog[0] ^= 1
                    return pool

                def proj_tile(cols):
                    pool = next_pool()
                    t = pool.tile(
                        [128, (4 if pool is qkA else 3) * 512],
                        F32,
                        tag="qk",
                        name="ps",
                    )
                    return t[:, 0:cols]

                def emit_vt4(kt0):
                    # four key tiles' vT in one staging slot, one strided copy
                    ps = proj_tile(4 * HID)
                    for t in range(4):
                        for ct in range(2):
                            nc.tensor.matmul(
                                ps[:, ts(t, HID)],
                                x_ap(ct, (kt0 + t) * 128, 128),
                                wv_sb[ct][:],
                                start=(ct == 0),
                                stop=(ct == 1),
                            )
                    dst = vT_sb[:, ds(kt0 * 132, 4 * 132)].rearrange(
                        "p (t h w) -> p t h w", t=4, w=33
                    )[:, :, :, 0:32]
                    src = ps.rearrange("p (t w) -> p t w", t=4).rearrange(
                        "p t (h w) -> p t h w", w=32
                    )
                    nc.vector.tensor_copy(dst, src)

                def emit_k(h, j):
                    ps = proj_tile(512)
                    for ct in range(2):
                        nc.tensor.matmul(
                            ps[:],
                            wkr_sb[ct][:, ts(h, HID)],
                            x_ap(ct, j * 512, 512),
                            start=(ct == 0),
                            stop=(ct == 1),
                        )
                    nc.vector.tensor_copy(krep[h][:, ts(j, 512)], ps[:])

                def emit_q(h, j):
                    ps = proj_tile(512)
                    for ct in range(2):
                        nc.tensor.matmul(
                            ps[:],
                            wqr_sb[ct][:, ts(h, HID)],
                            xq_ap(ct, j * 512, 512),
                            start=(ct == 0),
                            stop=(ct == 1),
                        )
                    nc.vector.tensor_copy(qrep[h][:, ts(j, 512)], ps[:])

                outh = [
                    osb.tile([HID, 512], F32R, tag=f"outh{c}", name=f"outh{c}")
                    for c in range(NCH)
                ]

                def emit_norm(h, ci, pv):
                    # rows 0..31 / row 32
                    pvs = norm_pool.tile([33, 512], F32, tag="pvs", name="pvs")
                    nc.vector.tensor_copy(pvs[:], pv[0:33, :])
                    rec = norm_pool.tile([1, 512], F32, tag="rec", name="rec")
                    nc.vector.reciprocal(rec[:], pvs[32:33, :])
                    # broadcast 1/denom to 32 partitions via DRAM bounce
                    rdr = dram_pool.tile([1, 512], F32, tag="rdr", name="rdr")
                    nc.sync.dma_start(out=rdr[:], in_=rec[:])
                    bc = norm_pool.tile([D, 512], F32, tag="bc", name="bc")
                    nc.sync.dma_start(
                        out=bc[:],
                        in_=bass.AP(
                            tensor=rdr.tensor,
                            offset=rdr.offset,
                            ap=[[0, D]] + [list(a) for a in rdr.ap[1:]],
                        ),
                    )
                    nc.vector.tensor_mul(
                        outh[ci][ds(32 * h, 32), :], pvs[0:32, :], bc[:]
                    )

                pending = []

                def emit_outproj(ci):
                    for ot in range(2):
                        op = pvp.tile([128, 512], F32, tag="pv", name="op")
                        nc.tensor.matmul(
                            op[:],
                            wo_sb[:, ts(ot, 128)],
                            outh[ci][:],
                            start=True,
                            stop=True,
                        )
                        ob = osb.tile([128, 512], F32, tag="ob", name="ob")
                        nc.vector.tensor_scalar_add(ob[:], op[:], bo_sb[ot][:])
                        nc.sync.dma_start(
                            out=out[ds(ot * 128, 128), ts(ci, 512)], in_=ob[:]
                        )

                def pop_pv():
                    probs, kt0, gsz, h, ci, pv = pending.pop(0)
                    for j in range(gsz):
                        nc.tensor.matmul(
                            pv[0:33, :],
                            vT_sb[:, ds((kt0 + j) * 132 + 33 * h, 33)],
                            probs[:, ts(j, 512)],
                            start=(kt0 + j == 0),
                            stop=(kt0 + j == NKT - 1),
                        )
                    if kt0 + gsz == NKT:
                        emit_norm(h, ci, pv)
                        if h == HEADS - 1:
                            emit_outproj(ci)

                # prologue: first projections
                emit_k(0, 0)
                emit_k(0, 1)
                emit_vt4(0)
                emit_q(0, 0)

                for h in range(HEADS):
                    for ci in range(NCH):
                        pv = pvp.tile([128, 512], F32, tag="pv", name="pv")
                        kt = 0
                        g = -1
                        while kt < NKT:
                            g += 1
                            pool = next_pool()
                            gsz = min(4 if pool is qkA else 3, NKT - kt)
                            qk = pool.tile(
                                [128, gsz * 512], F32, tag="qk", name="qkg"
                            )
                            for j in range(gsz):
                                band = (kt + j) % 4
                                nc.tensor.matmul(
                                    qk[:, ts(j, 512)],
                                    krep[h][ds(32 * band, 32), ts(kt + j, 128)],
                                    qrep[h][ds(32 * band, 32), ts(ci, 512)],
                                    start=True,
                                    stop=True,
                                    tile_position=(32 * band, 0),
                                )
                            probs = probs_pool.tile(
                                [128, gsz * 512], BF16, tag="pr", name="pr"
                            )
                            nc.scalar.activation(probs[:], qk[:], EXP)
                            pending.append((probs, kt, gsz, h, ci, pv))
                            if len(pending) > PVLAG:
                                pop_pv()
                            kt += gsz
                            # feed upcoming projections into PE idle slots
                            if ci == 0 and h == 0 and g < 7:
                                if g < 6:
                                    emit_k(h, g + 2)
                                if 4 * g + 4 < NKT:
                                    emit_vt4(4 * g + 4)
                            if ci == 0 and h > 0 and 2 <= g < 6:
                                emit_k(h, g + 2)
                            if g == 1 and ci < NCH - 1:
                                emit_q(h, ci + 1)
                            if ci == NCH - 1 and h < HEADS - 1 and 2 <= g < 6:
                                emit_k(h + 1, g - 2)
                                if g == 2:
                                    emit_q(h + 1, 0)
                while pending:
                    pop_pv()

    nc.finalize()
    return nc


_NC_CACHE = None


def make_in_maps(x, w_qkv, w_out, b_out):
    bf16 = ml_dtypes.bfloat16
    x = np.ascontiguousarray(np.asarray(x, dtype=np.float32)).reshape(4, C, N)
    w_qkv = np.asarray(w_qkv, dtype=np.float32)
    w_out = np.asarray(w_out, dtype=np.float32)
    b_out = np.asarray(b_out, dtype=np.float32)

    wqT = (w_qkv[0:HID] * SCALE).T                              # [256, 128]
    wkT = w_qkv[HID:2 * HID].T                                  # [256, 128]
    # per-head projection weights, head block replicated 4x along columns
    wqrT = np.ascontiguousarray(
        np.concatenate(
            [np.tile(wqT[:, 32 * h:32 * (h + 1)], (1, 4)) for h in range(HEADS)],
            axis=1,
        )
    ).astype(bf16)
    wkrT = np.ascontiguousarray(
        np.concatenate(
            [np.tile(wkT[:, 32 * h:32 * (h + 1)], (1, 4)) for h in range(HEADS)],
            axis=1,
        )
    ).astype(bf16)
    wvT = np.ascontiguousarray(w_qkv[2 * HID:3 * HID].T).astype(bf16)
    woT = np.ascontiguousarray(w_out.T)                         # [128, 256]
    boutc = np.ascontiguousarray(b_out.reshape(C, 1))
    xbf = x.astype(bf16)

    in_maps = []
    for core in range(NCORES):
        b, half = divmod(core, 2)
        in_maps.append(
            {
                "xb": xbf[b],
                "xq": np.ascontiguousarray(xbf[b][:, half * NQ:(half + 1) * NQ]),
                "wqrT": wqrT,
                "wkrT": wkrT,
                "wvT": wvT,
                "woT": woT,
                "bout": boutc,
            }
        )
    return in_maps


def kernel(x, w_qkv, w_out, b_out):
    global _NC_CACHE
    if _NC_CACHE is None:
        _NC_CACHE = build_nc()
    nc = _NC_CACHE
    in_maps = make_in_maps(x, w_qkv, w_out, b_out)
    res = run_bass_kernel_spmd(nc, in_maps, core_ids=list(range(NCORES)))
    out = np.empty((4, C, N), dtype=np.float32)
    for core in range(NCORES):
        b, half = divmod(core, 2)
        out[b][:, half * NQ:(half + 1) * NQ] = res.results[core]["out"]
    return out.reshape(4, C, 64, 64)


# revision 22
# speedup vs baseline: 1.2425x; 1.0576x over previous
"""Trainium2 Bass kernel for nn_Attention_21715354649378.

Reference computation (per batch b of 4):
    qkv = w_qkv @ x        x: [256, 4096(=64x64)]   w_qkv: [384, 256]
    q,k,v: [4 heads, 32, 4096];  q *= 32**-0.5
    sim_h = q_h^T k_h   [4096, 4096];  attn = softmax(sim, axis=-1)
    out_h = attn @ v_h^T    -> [4096, 32]
    out = w_out @ concat_heads + b_out   [256, 4096]

Sharding: 8 cores = 4 batches x 2 query-halves. Each core computes K/V for
its full batch plus attention + output projection for its half of the query
pixels. Outputs are disjoint slices -> no collectives.

Device algorithm per core (keys-in-partition layout, no max-subtraction --
sim values are O(6) so exp is safe in f32):
    vT = x^T W_v^T          per 128-key tile, 4 head blocks [v_h (32) | 1]
    krep_h = repl4(W_k,h) x   [128 = 4 copies of k_h(32d), 4096]  bf16
    qrep_h = repl4(s W_q,h) xq [128, 2048] bf16
      (replication lets QK use PE row-group kt%4: fast-weight-loads overlap
       and up to 4 concurrent matmuls in different 32-row bands)
    flat software pipeline over chunks (h, ci) and key-tile groups, using two
    alternating PSUM staging pools (4 + 3 banks) shared with the projection
    stream, PV lagging exp by two groups so activations run back-to-back:
        simT[kt] = krep_h[band, kt].T @ qrep_h[band, ci]   -> PSUM
        probs = exp(simT)     (ScalarE, PSUM->SBUF, bf16)
        pv += [v_h|1].T @ probs  (accumulate all 32 kt)    -> [33, 512]
    rows 0..31 = unnormalized out, row 32 = softmax denominator;
    outh[ci][32h:] = pv[0:32] * bcast(1/pv[32]) (recip + DRAM-bounce DMA)
    out[ci] = W_o @ outh[ci] + b_out  -> DMA out
"""

import numpy as np
import ml_dtypes

import concourse.bass as bass
import concourse.mybir as mybir
import concourse.tile as tile
from concourse import bacc
from concourse.bass import ts, ds
from concourse.bass_utils import run_bass_kernel_spmd

HEADS = 4
D = 32
HID = 128
C = 256
N = 4096
NQ = 2048
SCALE = D ** -0.5
NCORES = 8

F32 = mybir.dt.float32
F32R = mybir.dt.float32r
BF16 = mybir.dt.bfloat16
EXP = mybir.ActivationFunctionType.Exp

NKT = N // 128  # 32 key tiles per chunk
NCH = NQ // 512  # 4 query chunks
PVLAG = 2  # PV trails its exp by this many staging groups


def build_nc():
    nc = bacc.Bacc("TRN2")

    xb = nc.declare_dram_parameter("xb", [C, N], BF16, isOutput=False)
    xq = nc.declare_dram_parameter("xq", [C, NQ], BF16, isOutput=False)
    wqrT = nc.declare_dram_parameter("wqrT", [C, HEADS * HID], BF16, isOutput=False)
    wkrT = nc.declare_dram_parameter("wkrT", [C, HEADS * HID], BF16, isOutput=False)
    wvT = nc.declare_dram_parameter("wvT", [C, HID], BF16, isOutput=False)
    woT = nc.declare_dram_parameter("woT", [HID, C], F32R, isOutput=False)
    bout = nc.declare_dram_parameter("bout", [C, 1], F32, isOutput=False)
    out = nc.declare_dram_parameter("out", [C, NQ], F32, isOutput=True)

    with tile.TileContext(nc) as tc:
        with (
            nc.allow_low_precision(reason="bf16/fp32r attention core"),
            tc.tile_pool(name="persist", bufs=1) as persist,
            tc.tile_pool(name="wts", bufs=1) as wts,
            tc.tile_pool(name="dram", bufs=2, space="DRAM") as dram_pool,
        ):
            # ---- persistent SBUF tensors ----
            x_sb = [
                [
                    persist.tile([128, N // 4], BF16, tag=f"x{i}{j}", name=f"x{i}{j}")
                    for j in range(4)
                ]
                for i in range(2)
            ]
            xq_sb = [
                [
                    persist.tile([128, NQ // 2], BF16, tag=f"xq{i}{j}", name=f"xq{i}{j}")
                    for j in range(2)
                ]
                for i in range(2)
            ]
            krep = [
                persist.tile([128, N], BF16, tag=f"krep{h}", name=f"krep{h}")
                for h in range(HEADS)
            ]
            qrep = [
                persist.tile([128, NQ], BF16, tag=f"qrep{h}", name=f"qrep{h}")
                for h in range(HEADS)
            ]
            # per key-tile: 4 head blocks of [v_h (32) | ones (1)]
            vT_sb = persist.tile([128, NKT * 132], BF16, tag="vT")

            wqr_sb = [
                wts.tile([128, HEADS * HID], BF16, tag=f"wqr{i}", name=f"wqr{i}")
                for i in range(2)
            ]
            wkr_sb = [
                wts.tile([128, HEADS * HID], BF16, tag=f"wkr{i}", name=f"wkr{i}")
                for i in range(2)
            ]
            wv_sb = [
                wts.tile([128, HID], BF16, tag=f"wv{i}", name=f"wv{i}")
                for i in range(2)
            ]
            wo_sb = wts.tile([HID, C], F32R, tag="wo")
            bo_sb = [
                wts.tile([128, 1], F32, tag=f"bo{i}", name=f"bo{i}")
                for i in range(2)
            ]
            ones_sb = wts.tile([1, D], F32, tag="ones")

            # ---- DMA inputs, ordered by first use (~0.6us issue each) ----
            for i in range(2):
                nc.sync.dma_start(out=wkr_sb[i][:], in_=wkrT[ds(i * 128, 128), :])
            for i in range(2):
                nc.sync.dma_start(
                    out=x_sb[i][0][:], in_=xb[ds(i * 128, 128), ts(0, N // 4)]
                )
            for i in range(2):
                nc.sync.dma_start(out=wv_sb[i][:], in_=wvT[ds(i * 128, 128), :])
                nc.sync.dma_start(out=wqr_sb[i][:], in_=wqrT[ds(i * 128, 128), :])
            for i in range(2):
                nc.sync.dma_start(
                    out=xq_sb[i][0][:], in_=xq[ds(i * 128, 128), ts(0, NQ // 2)]
                )
            for j in range(1, 4):
                for i in range(2):
                    nc.sync.dma_start(
                        out=x_sb[i][j][:],
                        in_=xb[ds(i * 128, 128), ts(j, N // 4)],
                    )
            for i in range(2):
                nc.sync.dma_start(
                    out=xq_sb[i][1][:], in_=xq[ds(i * 128, 128), ts(1, NQ // 2)]
                )
                nc.sync.dma_start(out=bo_sb[i][:], in_=bout[ds(i * 128, 128), :])
            nc.sync.dma_start(out=wo_sb[:], in_=woT[:, :])
            nc.vector.memset(vT_sb[:], 1.0)
            nc.vector.memset(ones_sb[:], 1.0)

            with (
                tc.tile_pool(name="qkA", bufs=1, space="PSUM") as qkA,
                tc.tile_pool(name="qkB", bufs=1, space="PSUM") as qkB,
                tc.tile_pool(name="pvp", bufs=1, space="PSUM") as pvp,
                tc.tile_pool(name="probs", bufs=6) as probs_pool,
                tc.tile_pool(name="norm", bufs=3) as norm_pool,
                tc.tile_pool(name="osb", bufs=2) as osb,
            ):
                # staging slots rotate globally between the two pools;
                # projection tiles share the same rotation (no extra banks)
                _ptog = [0]

                def x_ap(ct, c0, length):
                    t_idx = c0 // (N // 4)
                    return x_sb[ct][t_idx][:, ds(c0 % (N // 4), length)]

                def xq_ap(ct, c0, length):
                    t_idx = c0 // (NQ // 2)
                    return xq_sb[ct][t_idx][:, ds(c0 % (NQ // 2), length)]

                def next_pool():
                    pool = qkA if _ptog[0] == 0 else qkB
                    _ptog[0] ^= 1
                    return pool

                def proj_tile(cols):
                    pool = next_pool()
                    t = pool.tile(
                        [128, (4 if pool is qkA else 3) * 512],
                        F32,
                        tag="qk",
                        name="ps",
                    )
                    return t[:, 0:cols]

                def emit_vt4(kt0):
                    # four key tiles' vT in one staging slot, one strided copy
                    ps = proj_tile(4 * HID)
                    for t in range(4):
                        for ct in range(2):
                            nc.tensor.matmul(
                                ps[:, ts(t, HID)],
                                x_ap(ct, (kt0 + t) * 128, 128),
                                wv_sb[ct][:],
                                start=(ct == 0),
                                stop=(ct == 1),
                            )
                    dst = vT_sb[:, ds(kt0 * 132, 4 * 132)].rearrange(
                        "p (t h w) -> p t h w", t=4, w=33
                    )[:, :, :, 0:32]
                    src = ps.rearrange("p (t w) -> p t w", t=4).rearrange(
                        "p t (h w) -> p t h w", w=32
                    )
                    nc.vector.tensor_copy(dst, src)

                def emit_k(h, j):
                    ps = proj_tile(512)
                    for ct in range(2):
                        nc.tensor.matmul(
                            ps[:],
                            wkr_sb[ct][:, ts(h, HID)],
                            x_ap(ct, j * 512, 512),
                            start=(ct == 0),
                            stop=(ct == 1),
                        )
                    nc.vector.tensor_copy(krep[h][:, ts(j, 512)], ps[:])

                def emit_q(h, j):
                    ps = proj_tile(512)
                    for ct in range(2):
                        nc.tensor.matmul(
                            ps[:],
                            wqr_sb[ct][:, ts(h, HID)],
                            xq_ap(ct, j * 512, 512),
                            start=(ct == 0),
                            stop=(ct == 1),
                        )
                    nc.vector.tensor_copy(qrep[h][:, ts(j, 512)], ps[:])

                outh = [
                    osb.tile([HID, 512], F32R, tag=f"outh{c}", name=f"outh{c}")
                    for c in range(NCH)
                ]

                def emit_norm(h, ci, pv):
                    # rows 0..31 / row 32
                    pvs = norm_pool.tile([33, 512], F32, tag="pvs", name="pvs")
                    nc.vector.tensor_copy(pvs[:], pv[0:33, :])
                    rec = norm_pool.tile([1, 512], F32, tag="rec", name="rec")
                    nc.vector.reciprocal(rec[:], pvs[32:33, :])
                    # broadcast 1/denom to 32 partitions via DRAM bounce
                    rdr = dram_pool.tile([1, 512], F32, tag="rdr", name="rdr")
                    nc.sync.dma_start(out=rdr[:], in_=rec[:])
                    bc = norm_pool.tile([D, 512], F32, tag="bc", name="bc")
                    nc.sync.dma_start(
                        out=bc[:],
                        in_=bass.AP(
                            tensor=rdr.tensor,
                            offset=rdr.offset,
                            ap=[[0, D]] + [list(a) for a in rdr.ap[1:]],
                        ),
                    )
                    nc.vector.tensor_mul(
                        outh[ci][ds(32 * h, 32), :], pvs[0:32, :], bc[:]
                    )

                pending = []
                deferred_op = []

                def emit_outproj(ci):
                    for ot in range(2):
                        op = pvp.tile([128, 512], F32, tag="pv", name="op")
                        nc.tensor.matmul(
                            op[:],
                            wo_sb[:, ts(ot, 128)],
                            outh[ci][:],
                            start=True,
                            stop=True,
                        )
                        ob = osb.tile([128, 512], F32, tag="ob", name="ob")
                        nc.vector.tensor_scalar_add(ob[:], op[:], bo_sb[ot][:])
                        nc.sync.dma_start(
                            out=out[ds(ot * 128, 128), ts(ci, 512)], in_=ob[:]
                        )

                def pop_pv():
                    probs, kt0, gsz, h, ci, pv = pending.pop(0)
                    for j in range(gsz):
                        nc.tensor.matmul(
                            pv[0:33, :],
                            vT_sb[:, ds((kt0 + j) * 132 + 33 * h, 33)],
                            probs[:, ts(j, 512)],
                            start=(kt0 + j == 0),
                            stop=(kt0 + j == NKT - 1),
                        )
                    if kt0 + gsz == NKT:
                        emit_norm(h, ci, pv)
                        if h == HEADS - 1:
                            deferred_op.append(ci)

                # prologue: first projections
                emit_k(0, 0)
                emit_k(0, 1)
                emit_vt4(0)
                emit_q(0, 0)

                for h in range(HEADS):
                    for ci in range(NCH):
                        pv = pvp.tile([128, 512], F32, tag="pv", name="pv")
                        kt = 0
                        g = -1
                        while kt < NKT:
                            g += 1
                            pool = next_pool()
                            gsz = min(4 if pool is qkA else 3, NKT - kt)
                            qk = pool.tile(
                                [128, gsz * 512], F32, tag="qk", name="qkg"
                            )
                            for j in range(gsz):
                                band = (kt + j) % 4
                                nc.tensor.matmul(
                                    qk[:, ts(j, 512)],
                                    krep[h][ds(32 * band, 32), ts(kt + j, 128)],
                                    qrep[h][ds(32 * band, 32), ts(ci, 512)],
                                    start=True,
                                    stop=True,
                                    tile_position=(32 * band, 0),
                                )
                            probs = probs_pool.tile(
                                [128, gsz * 512], BF16, tag="pr", name="pr"
                            )
                            nc.scalar.activation(probs[:], qk[:], EXP)
                            pending.append((probs, kt, gsz, h, ci, pv))
                            if len(pending) > PVLAG:
                                pop_pv()
                            if g == 4 and deferred_op:
                                emit_outproj(deferred_op.pop(0))
                            kt += gsz
                            # feed upcoming projections into PE idle slots
                            if ci == 0 and h == 0 and g < 7:
                                if g < 6:
                                    emit_k(h, g + 2)
                                if 4 * g + 4 < NKT:
                                    emit_vt4(4 * g + 4)
                            if ci == 0 and h > 0 and 2 <= g < 6:
                                emit_k(h, g + 2)
                            if g == 1 and ci < NCH - 1:
                                emit_q(h, ci + 1)
                            if ci == NCH - 1 and h < HEADS - 1 and 2 <= g < 6:
                                emit_k(h + 1, g - 2)
                                if g == 2:
                                    emit_q(h + 1, 0)
                while pending:
                    pop_pv()
                while deferred_op:
                    emit_outproj(deferred_op.pop(0))

    nc.finalize()
    return nc


_NC_CACHE = None


def make_in_maps(x, w_qkv, w_out, b_out):
    bf16 = ml_dtypes.bfloat16
    x = np.ascontiguousarray(np.asarray(x, dtype=np.float32)).reshape(4, C, N)
    w_qkv = np.asarray(w_qkv, dtype=np.float32)
    w_out = np.asarray(w_out, dtype=np.float32)
    b_out = np.asarray(b_out, dtype=np.float32)

    wqT = (w_qkv[0:HID] * SCALE).T                              # [256, 128]
    wkT = w_qkv[HID:2 * HID].T                                  # [256, 128]
    # per-head projection weights, head block replicated 4x along columns
    wqrT = np.ascontiguousarray(
        np.concatenate(
            [np.tile(wqT[:, 32 * h:32 * (h + 1)], (1, 4)) for h in range(HEADS)],
            axis=1,
        )
    ).astype(bf16)
    wkrT = np.ascontiguousarray(
        np.concatenate(
            [np.tile(wkT[:, 32 * h:32 * (h + 1)], (1, 4)) for h in range(HEADS)],
            axis=1,
        )
    ).astype(bf16)
    wvT = np.ascontiguousarray(w_qkv[2 * HID:3 * HID].T).astype(bf16)
    woT = np.ascontiguousarray(w_out.T)                         # [128, 256]
    boutc = np.ascontiguousarray(b_out.reshape(C, 1))
    xbf = x.astype(bf16)

    in_maps = []
    for core in range(NCORES):
        b, half = divmod(core, 2)
        in_maps.append(
            {
                "xb": xbf[b],
                "xq": np.ascontiguousarray(xbf[b][:, half * NQ:(half + 1) * NQ]),
                "wqrT": wqrT,
                "wkrT": wkrT,
                "wvT": wvT,
                "woT": woT,
                "bout": boutc,
            }
        )
    return in_maps


def kernel(x, w_qkv, w_out, b_out):
    global _NC_CACHE
    if _NC_CACHE is None:
        _NC_CACHE = build_nc()
    nc = _NC_CACHE
    in_maps = make_in_maps(x, w_qkv, w_out, b_out)
    res = run_bass_kernel_spmd(nc, in_maps, core_ids=list(range(NCORES)))
    out = np.empty((4, C, N), dtype=np.float32)
    for core in range(NCORES):
        b, half = divmod(core, 2)
        out[b][:, half * NQ:(half + 1) * NQ] = res.results[core]["out"]
    return out.reshape(4, C, 64, 64)
